# revision 2
# baseline (speedup 1.0000x reference)
"""Trainium2 Bass kernel for nn_Attention: 16-head attention, B=2, S=2048, H=1024.

Tensor-parallel over heads (8 cores x 2 heads), Megatron-style with the
all-reduce after dense done host-side at gather time.

v2 design (vs the 217us baseline):
  - bf16 everywhere on the matmul inputs (1.0 cycles/row at ANY output free
    size, and half the DMA bytes for X and weights). PSUM stays fp32.
  - v is produced directly in key-major [token, d] layout by using the
    resident XT chunks as matmul *weights* (contraction over H), eliminating
    the per-head PE transposes of the baseline.
  - ctx uses probs-as-weights orientation: out[q, d] = sum_k P[k,q] V[k,d]
    with ap_size 65 (64 dims + a ones column that yields the softmax row
    sums), halving ctx PE time vs streaming 512 queries per key chunk.
  - softmax normalization is deferred to a small DVE epilogue (reciprocal of
    the 65th ctx column) fused with the psum->sbuf copy; the normalized ctx
    block is PE-transposed back to [d, token] for the dense matmul.
  - qkv biases are added on the DVE (ACT does exp only); the v bias is folded
    into dense_b on the host (ctx = (ctx_raw + bv*rowsum)/rowsum).
  - dense partials are DMAed to DRAM straight from PSUM.
"""
import os
import numpy as np

B, S, H, NH = 2, 2048, 1024, 16
HD = H // NH            # 64
BS = B * S              # 4096
NCORES = 8
NK = H // 128           # 8 contraction chunks
NN = BS // 512          # 8 token blocks of 512
NKC = S // 128          # 16 key chunks per batch
NQC = 4                 # 128-query chunks per 512 query block

_CACHE = {}


def _build_program():
    import concourse.mybir as mybir
    import concourse.tile as tile
    from concourse import bacc

    F32 = mybir.dt.float32
    BF16 = mybir.dt.bfloat16
    Act = mybir.ActivationFunctionType

    nc = bacc.Bacc("TRN2", target_bir_lowering=False, debug=False,
                   num_devices=NCORES)
    xt = nc.dram_tensor("xt", [H, BS], BF16, kind="ExternalInput").ap()
    w1t = nc.dram_tensor("w1t", [H, 384], BF16, kind="ExternalInput").ap()
    b1 = nc.dram_tensor("b1", [128, 2], F32, kind="ExternalInput").ap()
    w2t = nc.dram_tensor("w2t", [128, H], BF16, kind="ExternalInput").ap()
    eye = nc.dram_tensor("eye", [128, 128], BF16, kind="ExternalInput").ap()
    ones16 = nc.dram_tensor("ones16", [128, 16], BF16, kind="ExternalInput").ap()
    out = nc.dram_tensor("out", [BS, H], F32, kind="ExternalOutput").ap()

    with tile.TileContext(nc) as tc, nc.allow_low_precision(reason="bf16"):
        from contextlib import ExitStack
        with ExitStack() as ctx:
            consts = ctx.enter_context(tc.tile_pool(name="consts", bufs=1))
            bigs = ctx.enter_context(tc.tile_pool(name="bigs", bufs=1))
            xtp = ctx.enter_context(tc.tile_pool(name="xtp", bufs=5))
            etp = ctx.enter_context(tc.tile_pool(name="etp", bufs=24))
            cnp = ctx.enter_context(tc.tile_pool(name="cnp", bufs=8))
            rp = ctx.enter_context(tc.tile_pool(name="rp", bufs=6))
            outs = ctx.enter_context(tc.tile_pool(name="outs", bufs=3))
            ps_sc = ctx.enter_context(tc.tile_pool(name="ps_sc", bufs=2, space="PSUM"))
            ps_cx = ctx.enter_context(tc.tile_pool(name="ps_cx", bufs=2, space="PSUM"))
            ps_ms = ctx.enter_context(tc.tile_pool(name="ps_ms", bufs=2, space="PSUM"))

            # ---- constants (first xt block's weights before anything else
            # so the first qkv matmul can start ~2.5us in) ----
            w1big = consts.tile([128, NK, 384], BF16, name="w1big")
            w1r = w1t.rearrange("(k p) r -> p k r", p=128)
            nc.sync.dma_start(w1big[:, 0:1, :], w1r[:, 0:1, :])

            def emit_consts_rest():
                warm = consts.tile([1, 1], F32, name="warm")
                nc.scalar.activation(warm[0:1, 0:1], w1big[0:1, 0, 0:1], Act.Exp)
                nc.sync.dma_start(w1big[:, 1:NK // 2, :], w1r[:, 1:NK // 2, :])
                nc.sync.dma_start(w1big[:, NK // 2:NK, :], w1r[:, NK // 2:NK, :])
                b1sb = consts.tile([128, 2], F32, name="b1")
                nc.sync.dma_start(b1sb[:], b1)
                eyesb = consts.tile([128, 128], BF16, name="eye")
                nc.sync.dma_start(eyesb[:], eye)
                onesb = consts.tile([128, 16], BF16, name="ones16")
                nc.sync.dma_start(onesb[:], ones16)
                w2sb = consts.tile([128, H], BF16, name="w2")
                nc.sync.dma_start(w2sb[:], w2t)
                return b1sb, eyesb, onesb, w2sb

            KG = 4  # k-chunks per xt DMA

            def emit_qkv_dma(n, fine=False):
                xts = []
                for kg in range(NK // KG):
                    xt_t = xtp.tile([128, KG, 512], BF16, name="xt")
                    if fine:
                        for c in range(KG):
                            k = kg * KG + c
                            nc.sync.dma_start(
                                xt_t[:, c, :],
                                xt[k * 128:(k + 1) * 128,
                                   n * 512:(n + 1) * 512])
                    else:
                        nc.sync.dma_start(
                            xt_t[:],
                            xt[kg * KG * 128:(kg + 1) * KG * 128,
                               n * 512:(n + 1) * 512].rearrange(
                                   "(c p) f -> p c f", p=128))
                    xts.append(xt_t)
                return xts

            qkv_ps = {}

            def emit_qkv_m(n, xts, m, half=None):
                """q (m=0) or k (m=1) rows for token block n, bias on DVE.
                half=0/1 emits only the low/high contraction half (the psum
                accumulation group stays open in between) so a ladder slot
                never inserts more than ~0.9us of PE work."""
                dst = [qt, kt]
                if half in (None, 0):
                    qkv_ps[(n, m)] = ps_ms.tile([128, 512], F32,
                                                name=f"qk{m}", tag="ms")
                ps = qkv_ps[(n, m)]
                ks = range(NK) if half is None else \
                    range(half * NK // 2, (half + 1) * NK // 2)
                for k in ks:
                    nc.tensor.matmul(
                        ps[:],
                        w1big[:, k, m * 128:(m + 1) * 128],
                        xts[k // KG][:, k % KG, :],
                        start=(k == 0), stop=(k == NK - 1))
                if half in (None, 1):
                    nc.vector.tensor_scalar_add(
                        dst[m][:, n * 512:(n + 1) * 512], ps[:],
                        b1sb[:, m:m + 1])

            def emit_qkv_v(n, xts, t4):
                """v-direct [token, d] chunk t4 of block n -> vbig (gpsimd)."""
                b = n // 4
                kc = (n % 4) * 4 + t4
                vp = ps_ms.tile([128, 128], F32, name="vd", tag="ms")
                for k in range(NK):
                    nc.tensor.matmul(
                        vp[:],
                        xts[k // KG][:, k % KG, t4 * 128:(t4 + 1) * 128],
                        w1big[:, k, 256:384],
                        start=(k == 0), stop=(k == NK - 1))
                for j in range(2):
                    nc.vector.tensor_copy(
                        vbig[(b, j)][:, kc * (HD + 1):kc * (HD + 1) + HD],
                        vp[:, j * HD:(j + 1) * HD])

            def emit_qkv_compute(n, xts):
                emit_qkv_m(n, xts, 0)
                emit_qkv_m(n, xts, 1)
                for t4 in range(4):
                    emit_qkv_v(n, xts, t4)

            def phase_a_pieces(n, xts, fine=False):
                """PE pieces for laddering block n into a window without
                starving the exp stream. Coarse: 4 pieces of ~0.9-1.7us.
                Fine: 8 pieces of ~0.4-0.9us; the qkv halves of a pair must
                occupy adjacent slots (no other ps_ms alloc in between)."""
                if not fine:
                    return [
                        lambda: emit_qkv_m(n, xts, 0),
                        lambda: emit_qkv_m(n, xts, 1),
                        lambda: (emit_qkv_v(n, xts, 0), emit_qkv_v(n, xts, 1)),
                        lambda: (emit_qkv_v(n, xts, 2), emit_qkv_v(n, xts, 3)),
                    ]
                return [
                    lambda: emit_qkv_m(n, xts, 0, half=0),
                    lambda: emit_qkv_m(n, xts, 0, half=1),
                    lambda: emit_qkv_m(n, xts, 1, half=0),
                    lambda: emit_qkv_m(n, xts, 1, half=1),
                    lambda: emit_qkv_v(n, xts, 0),
                    lambda: emit_qkv_v(n, xts, 1),
                    lambda: emit_qkv_v(n, xts, 2),
                    lambda: emit_qkv_v(n, xts, 3),
                ]

            def emit_scores_exp(b, qb, kc):
                sp = ps_sc.tile([128, 1024], F32, name="scores", tag="sc")
                for j in range(2):
                    nc.tensor.matmul(
                        sp[:, j * 512:(j + 1) * 512],
                        kt[64 * j:64 * j + 64,
                           b * S + kc * 128:b * S + (kc + 1) * 128],
                        qt[64 * j:64 * j + 64,
                           b * S + qb * 512:b * S + (qb + 1) * 512],
                        start=True, stop=True)
                et = etp.tile([128, 1024], BF16, name="exp")
                nc.scalar.activation(et[:], sp[:], Act.Exp, scale=0.125)
                return et

            def emit_ctx_group(pb, pq, j, qc, ets_prev):
                """One ctx accumulation group (one open psum group at a time
                per bank: kc must be the inner loop), then its normalize /
                transpose / cts epilogue."""
                cxp = ps_cx.tile([128, HD + 1], F32, name=f"cx{j}{qc}",
                                 tag="cx")
                for kc in range(NKC):
                    nc.tensor.matmul(
                        cxp[:],
                        ets_prev[kc][:, j * 512 + qc * 128:
                                     j * 512 + (qc + 1) * 128],
                        vbig[(pb, j)][:, kc * (HD + 1):(kc + 1) * (HD + 1)],
                        start=(kc == 0), stop=(kc == NKC - 1))
                r = rp.tile([128, 1], F32, name="recip")
                nc.vector.reciprocal_approx_fast(r[:], cxp[:, HD:HD + 1])
                c = cnp.tile([128, HD], BF16, name="cn")
                nc.vector.tensor_scalar_mul(c[:], cxp[:, 0:HD], r[:])
                tp = ps_ms.tile([HD, 128], BF16, name="ctxT", tag="ms")
                nc.tensor.transpose(tp[:], c[:], eyesb[:])
                nc.vector.tensor_copy(
                    cts[pb][64 * j:64 * j + 64,
                            pq * 512 + qc * 128:pq * 512 + (qc + 1) * 128],
                    tp[:])

            def emit_dense_t(b, t, tail=False):
                """Dense partial for 128-token chunk t of batch b; psum->sbuf
                staging runs on the otherwise-idle gpsimd engine (on ACT for
                the tail, where exp is finished but gpsimd would serialize)."""
                ob = outs.tile([128, H], F32, name="ostage")
                for nb in range(2):
                    dp = ps_ms.tile([128, 512], F32, name="dense", tag="ms")
                    nc.tensor.matmul(
                        dp[:], cts[b][:, t * 128:(t + 1) * 128],
                        w2sb[:, nb * 512:(nb + 1) * 512],
                        start=True, stop=True)
                    if tail:
                        nc.scalar.activation(
                            ob[:, nb * 512:(nb + 1) * 512], dp[:],
                            Act.Identity)
                    else:
                        nc.vector.tensor_copy(
                            ob[:, nb * 512:(nb + 1) * 512], dp[:])
                row0 = b * S + t * 128
                nc.sync.dma_start(out[row0:row0 + 128, :], ob[:])

            # ---- emission schedule ----
            # Startup: first xt block DMA right after the first w1big slice,
            # then the remaining consts. Phase A b0 (blocks 0-3), then 8
            # attention windows; batch-1 phase-A blocks ride windows 0-3.
            # Dense for b0 is deferred into the b1 windows (which have PE
            # slack, being exp-paced); only dense(1,3) remains as tail.
            xts0 = emit_qkv_dma(0)
            b1sb, eyesb, onesb, w2sb = emit_consts_rest()

            qt = bigs.tile([128, BS], BF16, name="qt")
            kt = bigs.tile([128, BS], BF16, name="kt")
            # vbig[b][j]: [128 keypos, 16 kc * (64 v + 1 ones)] bf16
            vbig = {(b, j): bigs.tile([128, NKC * (HD + 1)], BF16,
                                      name=f"vbig{b}{j}")
                    for b in range(2) for j in range(2)}
            cts = {b: bigs.tile([128, S], BF16, name=f"cts{b}")
                   for b in range(2)}
            for vb in range(2):
                for j in range(2):
                    ones_view = vbig[(vb, j)][:].rearrange(
                        "p (c w) -> p c w", w=HD + 1)[:, :, HD:HD + 1]
                    nc.vector.tensor_copy(ones_view, onesb[:, 0:NKC])

            emit_qkv_compute(0, xts0)
            xts_n1 = emit_qkv_dma(1)

            # Window w emits its own scores+exp per kc, the ctx groups of
            # window w-1 at kc 0-7 (one per kc, each a closed psum group),
            # phase-A pieces / dense chunks at kc 8-15. Window 0 absorbs
            # phase-A blocks 1-3; n4-n7 ride windows 1-4. dense(b,qb) runs
            # two windows after (b,qb), when its cts is complete.
            order = [(0, 0), (0, 1), (0, 2), (0, 3),
                     (1, 0), (1, 1), (1, 2), (1, 3)]
            wdense = {
                2: [(0, 0, 0), (0, 0, 1), (0, 0, 2), (0, 0, 3)],
                3: [(0, 1, 0), (0, 1, 1), (0, 1, 2), (0, 1, 3)],
                4: [(0, 2, 0), (0, 2, 1), (0, 2, 2), (0, 2, 3)],
                5: [(0, 3, 0), (0, 3, 1), (0, 3, 2), (0, 3, 3)],
                6: [(1, 0, 0), (1, 0, 1), (1, 0, 2), (1, 0, 3)],
                7: [(1, 1, 0), (1, 1, 1), (1, 1, 2), (1, 1, 3),
                    (1, 2, 0), (1, 2, 1), (1, 2, 2), (1, 2, 3)],
            }
            prev = None  # (b, qb, ets)
            pending_xts = {1: xts_n1}
            for w, (b, qb) in enumerate(order):
                piece_slots = {}   # kc -> list of phase-A piece indices
                dma_slots = {}
                if w == 0:
                    for i, n in enumerate((1, 2, 3)):
                        for q in range(4):
                            piece_slots[4 * i + q] = (n, q, False)
                    dma_slots = {0: 2, 4: 3, 8: 4}
                elif w <= 4:
                    # fine pieces at kc 8-15: qkv halves adjacent (8,9) and
                    # (11,12); dense pairs at 10 and 13 between closed groups
                    for i, kc_slot in enumerate((8, 9, 11, 12, 14, 15)):
                        piece_slots[kc_slot] = (3 + w, i, True)
                    if w < 4:
                        dma_slots = {6: 4 + w}
                chunks = [(db, dq * 4 + t4) for db, dq, t4 in wdense.get(w, [])]
                if 1 <= w <= 4:
                    dt_slots = {10: chunks[0:2], 13: chunks[2:4]}
                else:
                    dt_slots = {8 + i: [c] for i, c in enumerate(chunks)}

                ets = {}
                for kc in range(NKC):
                    if prev is not None and kc < 8:
                        pb, pq, pets = prev
                        emit_ctx_group(pb, pq, kc // 4, kc % 4, pets)
                    if kc in piece_slots:
                        # before this kc's scores: window 4's late scores
                        # read kt written by the same-slot n7 piece
                        pn, pq_, fine = piece_slots[kc]
                        pcs = phase_a_pieces(pn, pending_xts[pn], fine=fine)
                        if fine:
                            # 6 slots: m halves (0,1),(2,3) then v pairs
                            fmap = [[0], [1], [2], [3], [4, 5], [6, 7]]
                            for pi in fmap[pq_]:
                                pcs[pi]()
                        else:
                            pcs[pq_]()
                    ets[kc] = emit_scores_exp(b, qb, kc)
                    for ch in dt_slots.get(kc, []):
                        emit_dense_t(*ch)
                    if kc in dma_slots:
                        pending_xts[dma_slots[kc]] = emit_qkv_dma(dma_slots[kc])
                prev = (b, qb, ets)

            pb, pq, pets = prev
            for g in range(8):
                emit_ctx_group(pb, pq, g // 4, g % 4, pets)
            for t4 in range(4):
                emit_dense_t(pb, pq * 4 + t4, tail=True)
    nc.compile()
    return nc


def _to_bf16(x):
    import ml_dtypes
    return np.ascontiguousarray(x, dtype=np.float32).astype(ml_dtypes.bfloat16)


def _prepare_inputs(hidden_states, qkv_w, qkv_b, dense_w):
    """Host-side slicing/transposition into per-core input maps."""
    x = np.ascontiguousarray(hidden_states, dtype=np.float32).reshape(BS, H)
    xt = _to_bf16(x.T)
    eye = np.eye(128, dtype=np.float32)
    ones16 = np.ones((128, 16), dtype=np.float32)
    in_maps = []
    for c in range(NCORES):
        h0, h1 = 2 * c, 2 * c + 1
        rows = {}
        for m in range(3):  # 0=q 1=k 2=v; per-head groups of 192 rows
            rows[m] = np.r_[h0 * 192 + m * HD:h0 * 192 + (m + 1) * HD,
                            h1 * 192 + m * HD:h1 * 192 + (m + 1) * HD]
        perm = np.concatenate([rows[0], rows[1], rows[2]])
        w1tc = _to_bf16(qkv_w[perm, :].T)                    # [H, 384]
        b1c = np.ascontiguousarray(
            np.stack([qkv_b[rows[m]] for m in range(2)], axis=1),
            dtype=np.float32)                                # [128, 2]
        w2tc = _to_bf16(dense_w[:, c * 128:(c + 1) * 128].T)  # [128, H]
        in_maps.append({
            "xt": xt, "w1t": w1tc, "b1": b1c, "w2t": w2tc,
            "eye": _to_bf16(eye), "ones16": _to_bf16(ones16),
        })
    return in_maps


def _reference_numpy(hidden_states, attention_mask, qkv_w, qkv_b, dense_w, dense_b):
    """Exact fallback for non-all-ones masks (never hit with spec inputs)."""
    x = np.asarray(hidden_states, dtype=np.float64)
    mask = np.asarray(attention_mask, dtype=np.float64)
    mixed = x @ np.asarray(qkv_w, np.float64).T + np.asarray(qkv_b, np.float64)
    mixed = mixed.reshape(B, S, NH, 3 * HD).transpose(0, 2, 1, 3)
    q, k, v = np.split(mixed, 3, axis=-1)
    scores = np.einsum("bhqd,bhkd->bhqk", q, k) / np.sqrt(HD)
    scores = scores * mask - 10000.0 * (1.0 - mask)
    scores -= scores.max(axis=-1, keepdims=True)
    probs = np.exp(scores)
    probs /= probs.sum(axis=-1, keepdims=True)
    cx = np.einsum("bhqk,bhkd->bhqd", probs, v)
    cx = cx.transpose(0, 2, 1, 3).reshape(B, S, H)
    o = cx @ np.asarray(dense_w, np.float64).T + np.asarray(dense_b, np.float64)
    return o.astype(np.float32)


def _run(inputs, trace=False):
    from concourse.bass_utils import run_bass_kernel_spmd
    if "nc" not in _CACHE:
        _CACHE["nc"] = _build_program()
    nc = _CACHE["nc"]
    in_maps = _prepare_inputs(inputs["hidden_states"], inputs["qkv_w"],
                              inputs["qkv_b"], inputs["dense_w"])
    res = run_bass_kernel_spmd(nc, in_maps, core_ids=list(range(NCORES)),
                               trace=trace)
    partials = np.stack([r["out"] for r in res.results], axis=0)
    full = partials.sum(axis=0, dtype=np.float64)
    # v bias folds into the output bias: ctx = (ctx_raw + bv * rowsum)/rowsum
    qkv_b = np.asarray(inputs["qkv_b"], dtype=np.float64)
    bv = qkv_b.reshape(NH, 3 * HD)[:, 2 * HD:3 * HD].reshape(H)  # ctx col order
    bias_eff = np.asarray(inputs["dense_b"], dtype=np.float64) \
        + np.asarray(inputs["dense_w"], dtype=np.float64) @ bv
    full += bias_eff
    return full.astype(np.float32).reshape(B, S, H), res


def kernel(hidden_states, attention_mask, qkv_w, qkv_b, dense_w, dense_b):
    hidden_states = np.asarray(hidden_states)
    attention_mask = np.asarray(attention_mask)
    qkv_w = np.asarray(qkv_w)
    qkv_b = np.asarray(qkv_b)
    dense_w = np.asarray(dense_w)
    dense_b = np.asarray(dense_b)
    if not np.all(attention_mask == 1.0):
        return _reference_numpy(hidden_states, attention_mask, qkv_w, qkv_b,
                                dense_w, dense_b)
    out, _ = _run({
        "hidden_states": hidden_states, "qkv_w": qkv_w, "qkv_b": qkv_b,
        "dense_w": dense_w, "dense_b": dense_b,
    }, trace=bool(int(os.environ.get("KERNEL_TRACE", "0"))))
    return out


# revision 3
# speedup vs baseline: 1.0413x; 1.0413x over previous
"""Trainium2 Bass kernel for nn_Attention: 16-head attention, B=2, S=2048, H=1024.

Tensor-parallel over heads (8 cores x 2 heads), Megatron-style with the
all-reduce after dense done host-side at gather time.

v2 design (vs the 217us baseline):
  - bf16 everywhere on the matmul inputs (1.0 cycles/row at ANY output free
    size, and half the DMA bytes for X and weights). PSUM stays fp32.
  - v is produced directly in key-major [token, d] layout by using the
    resident XT chunks as matmul *weights* (contraction over H), eliminating
    the per-head PE transposes of the baseline.
  - ctx uses probs-as-weights orientation: out[q, d] = sum_k P[k,q] V[k,d]
    with ap_size 65 (64 dims + a ones column that yields the softmax row
    sums), halving ctx PE time vs streaming 512 queries per key chunk.
  - softmax normalization is deferred to a small DVE epilogue (reciprocal of
    the 65th ctx column) fused with the psum->sbuf copy; the normalized ctx
    block is PE-transposed back to [d, token] for the dense matmul.
  - qkv biases are added on the DVE (ACT does exp only); the v bias is folded
    into dense_b on the host (ctx = (ctx_raw + bv*rowsum)/rowsum).
  - dense partials are DMAed to DRAM straight from PSUM.
"""
import os
import numpy as np

B, S, H, NH = 2, 2048, 1024, 16
HD = H // NH            # 64
BS = B * S              # 4096
NCORES = 8
NK = H // 128           # 8 contraction chunks
NN = BS // 512          # 8 token blocks of 512
NKC = S // 128          # 16 key chunks per batch
NQC = 4                 # 128-query chunks per 512 query block

_CACHE = {}


def _build_program():
    import concourse.mybir as mybir
    import concourse.tile as tile
    from concourse import bacc

    F32 = mybir.dt.float32
    BF16 = mybir.dt.bfloat16
    Act = mybir.ActivationFunctionType

    nc = bacc.Bacc("TRN2", target_bir_lowering=False, debug=False,
                   num_devices=NCORES)
    xt = nc.dram_tensor("xt", [H, BS], BF16, kind="ExternalInput").ap()
    w1t = nc.dram_tensor("w1t", [H, 384], BF16, kind="ExternalInput").ap()
    b1 = nc.dram_tensor("b1", [128, 2], F32, kind="ExternalInput").ap()
    w2t = nc.dram_tensor("w2t", [128, H], BF16, kind="ExternalInput").ap()
    eye = nc.dram_tensor("eye", [128, 128], BF16, kind="ExternalInput").ap()
    ones16 = nc.dram_tensor("ones16", [128, 16], BF16, kind="ExternalInput").ap()
    out = nc.dram_tensor("out", [BS, H], BF16, kind="ExternalOutput").ap()

    with tile.TileContext(nc) as tc, nc.allow_low_precision(reason="bf16"):
        from contextlib import ExitStack
        with ExitStack() as ctx:
            consts = ctx.enter_context(tc.tile_pool(name="consts", bufs=1))
            bigs = ctx.enter_context(tc.tile_pool(name="bigs", bufs=1))
            xtp = ctx.enter_context(tc.tile_pool(name="xtp", bufs=5))
            etp = ctx.enter_context(tc.tile_pool(name="etp", bufs=24))
            cnp = ctx.enter_context(tc.tile_pool(name="cnp", bufs=8))
            rp = ctx.enter_context(tc.tile_pool(name="rp", bufs=6))
            outs = ctx.enter_context(tc.tile_pool(name="outs", bufs=3))
            ps_sc = ctx.enter_context(tc.tile_pool(name="ps_sc", bufs=2, space="PSUM"))
            ps_cx = ctx.enter_context(tc.tile_pool(name="ps_cx", bufs=2, space="PSUM"))
            ps_ms = ctx.enter_context(tc.tile_pool(name="ps_ms", bufs=2, space="PSUM"))

            # ---- constants (first xt block's weights before anything else
            # so the first qkv matmul can start ~2.5us in) ----
            w1big = consts.tile([128, NK, 384], BF16, name="w1big")
            w1r = w1t.rearrange("(k p) r -> p k r", p=128)
            nc.sync.dma_start(w1big[:, 0:1, :], w1r[:, 0:1, :])

            def emit_consts_rest():
                warm = consts.tile([1, 1], F32, name="warm")
                nc.scalar.activation(warm[0:1, 0:1], w1big[0:1, 0, 0:1], Act.Exp)
                nc.sync.dma_start(w1big[:, 1:NK // 2, :], w1r[:, 1:NK // 2, :])
                nc.sync.dma_start(w1big[:, NK // 2:NK, :], w1r[:, NK // 2:NK, :])
                b1sb = consts.tile([128, 2], F32, name="b1")
                nc.sync.dma_start(b1sb[:], b1)
                eyesb = consts.tile([128, 128], BF16, name="eye")
                nc.sync.dma_start(eyesb[:], eye)
                onesb = consts.tile([128, 16], BF16, name="ones16")
                nc.sync.dma_start(onesb[:], ones16)
                w2sb = consts.tile([128, H], BF16, name="w2")
                nc.sync.dma_start(w2sb[:], w2t)
                return b1sb, eyesb, onesb, w2sb

            KG = 4  # k-chunks per xt DMA

            def emit_qkv_dma(n, fine=False):
                xts = []
                for kg in range(NK // KG):
                    xt_t = xtp.tile([128, KG, 512], BF16, name="xt")
                    if fine:
                        for c in range(KG):
                            k = kg * KG + c
                            nc.sync.dma_start(
                                xt_t[:, c, :],
                                xt[k * 128:(k + 1) * 128,
                                   n * 512:(n + 1) * 512])
                    else:
                        nc.sync.dma_start(
                            xt_t[:],
                            xt[kg * KG * 128:(kg + 1) * KG * 128,
                               n * 512:(n + 1) * 512].rearrange(
                                   "(c p) f -> p c f", p=128))
                    xts.append(xt_t)
                return xts

            qkv_ps = {}

            def emit_qkv_m(n, xts, m, half=None):
                """q (m=0) or k (m=1) rows for token block n, bias on DVE.
                half=0/1 emits only the low/high contraction half (the psum
                accumulation group stays open in between) so a ladder slot
                never inserts more than ~0.9us of PE work."""
                dst = [qt, kt]
                if half in (None, 0):
                    qkv_ps[(n, m)] = ps_ms.tile([128, 512], F32,
                                                name=f"qk{m}", tag="ms")
                ps = qkv_ps[(n, m)]
                ks = range(NK) if half is None else \
                    range(half * NK // 2, (half + 1) * NK // 2)
                for k in ks:
                    nc.tensor.matmul(
                        ps[:],
                        w1big[:, k, m * 128:(m + 1) * 128],
                        xts[k // KG][:, k % KG, :],
                        start=(k == 0), stop=(k == NK - 1))
                if half in (None, 1):
                    nc.vector.tensor_scalar_add(
                        dst[m][:, n * 512:(n + 1) * 512], ps[:],
                        b1sb[:, m:m + 1])

            def emit_qkv_v(n, xts, t4):
                """v-direct [token, d] chunk t4 of block n -> vbig (gpsimd)."""
                b = n // 4
                kc = (n % 4) * 4 + t4
                vp = ps_ms.tile([128, 128], F32, name="vd", tag="ms")
                for k in range(NK):
                    nc.tensor.matmul(
                        vp[:],
                        xts[k // KG][:, k % KG, t4 * 128:(t4 + 1) * 128],
                        w1big[:, k, 256:384],
                        start=(k == 0), stop=(k == NK - 1))
                for j in range(2):
                    nc.vector.tensor_copy(
                        vbig[(b, j)][:, kc * (HD + 1):kc * (HD + 1) + HD],
                        vp[:, j * HD:(j + 1) * HD])

            def emit_qkv_compute(n, xts):
                emit_qkv_m(n, xts, 0)
                emit_qkv_m(n, xts, 1)
                for t4 in range(4):
                    emit_qkv_v(n, xts, t4)

            def phase_a_pieces(n, xts, fine=False):
                """PE pieces for laddering block n into a window without
                starving the exp stream. Coarse: 4 pieces of ~0.9-1.7us.
                Fine: 8 pieces of ~0.4-0.9us; the qkv halves of a pair must
                occupy adjacent slots (no other ps_ms alloc in between)."""
                if not fine:
                    return [
                        lambda: emit_qkv_m(n, xts, 0),
                        lambda: emit_qkv_m(n, xts, 1),
                        lambda: (emit_qkv_v(n, xts, 0), emit_qkv_v(n, xts, 1)),
                        lambda: (emit_qkv_v(n, xts, 2), emit_qkv_v(n, xts, 3)),
                    ]
                return [
                    lambda: emit_qkv_m(n, xts, 0, half=0),
                    lambda: emit_qkv_m(n, xts, 0, half=1),
                    lambda: emit_qkv_m(n, xts, 1, half=0),
                    lambda: emit_qkv_m(n, xts, 1, half=1),
                    lambda: emit_qkv_v(n, xts, 0),
                    lambda: emit_qkv_v(n, xts, 1),
                    lambda: emit_qkv_v(n, xts, 2),
                    lambda: emit_qkv_v(n, xts, 3),
                ]

            def emit_scores_exp(b, qb, kc):
                sp = ps_sc.tile([128, 1024], F32, name="scores", tag="sc")
                for j in range(2):
                    nc.tensor.matmul(
                        sp[:, j * 512:(j + 1) * 512],
                        kt[64 * j:64 * j + 64,
                           b * S + kc * 128:b * S + (kc + 1) * 128],
                        qt[64 * j:64 * j + 64,
                           b * S + qb * 512:b * S + (qb + 1) * 512],
                        start=True, stop=True)
                et = etp.tile([128, 1024], BF16, name="exp")
                nc.scalar.activation(et[:], sp[:], Act.Exp, scale=0.125)
                return et

            def emit_ctx_group(pb, pq, j, qc, ets_prev):
                """One ctx accumulation group (one open psum group at a time
                per bank: kc must be the inner loop), then its normalize /
                transpose / cts epilogue."""
                cxp = ps_cx.tile([128, HD + 1], F32, name=f"cx{j}{qc}",
                                 tag="cx")
                for kc in range(NKC):
                    nc.tensor.matmul(
                        cxp[:],
                        ets_prev[kc][:, j * 512 + qc * 128:
                                     j * 512 + (qc + 1) * 128],
                        vbig[(pb, j)][:, kc * (HD + 1):(kc + 1) * (HD + 1)],
                        start=(kc == 0), stop=(kc == NKC - 1))
                r = rp.tile([128, 1], F32, name="recip")
                nc.vector.reciprocal_approx_fast(r[:], cxp[:, HD:HD + 1])
                c = cnp.tile([128, HD], BF16, name="cn")
                nc.vector.tensor_scalar_mul(c[:], cxp[:, 0:HD], r[:])
                tp = ps_ms.tile([HD, 128], BF16, name="ctxT", tag="ms")
                nc.tensor.transpose(tp[:], c[:], eyesb[:])
                nc.vector.tensor_copy(
                    cts[pb][64 * j:64 * j + 64,
                            pq * 512 + qc * 128:pq * 512 + (qc + 1) * 128],
                    tp[:])

            def emit_dense_t(b, t, tail=False):
                """Dense partial for 128-token chunk t of batch b; psum->sbuf
                staging runs on the otherwise-idle gpsimd engine (on ACT for
                the tail, where exp is finished but gpsimd would serialize)."""
                ob = outs.tile([128, H], BF16, name="ostage")
                for nb in range(2):
                    dp = ps_ms.tile([128, 512], F32, name="dense", tag="ms")
                    nc.tensor.matmul(
                        dp[:], cts[b][:, t * 128:(t + 1) * 128],
                        w2sb[:, nb * 512:(nb + 1) * 512],
                        start=True, stop=True)
                    if tail and nb == 0:
                        nc.scalar.activation(
                            ob[:, nb * 512:(nb + 1) * 512], dp[:],
                            Act.Identity)
                    else:
                        nc.vector.tensor_copy(
                            ob[:, nb * 512:(nb + 1) * 512], dp[:])
                row0 = b * S + t * 128
                nc.sync.dma_start(out[row0:row0 + 128, :], ob[:])

            # ---- emission schedule ----
            # Startup: first xt block DMA right after the first w1big slice,
            # then the remaining consts. Phase A b0 (blocks 0-3), then 8
            # attention windows; batch-1 phase-A blocks ride windows 0-3.
            # Dense for b0 is deferred into the b1 windows (which have PE
            # slack, being exp-paced); only dense(1,3) remains as tail.
            xts0 = emit_qkv_dma(0)
            b1sb, eyesb, onesb, w2sb = emit_consts_rest()

            qt = bigs.tile([128, BS], BF16, name="qt")
            kt = bigs.tile([128, BS], BF16, name="kt")
            # vbig[b][j]: [128 keypos, 16 kc * (64 v + 1 ones)] bf16
            vbig = {(b, j): bigs.tile([128, NKC * (HD + 1)], BF16,
                                      name=f"vbig{b}{j}")
                    for b in range(2) for j in range(2)}
            cts = {b: bigs.tile([128, S], BF16, name=f"cts{b}")
                   for b in range(2)}
            for vb in range(2):
                for j in range(2):
                    ones_view = vbig[(vb, j)][:].rearrange(
                        "p (c w) -> p c w", w=HD + 1)[:, :, HD:HD + 1]
                    nc.vector.tensor_copy(ones_view, onesb[:, 0:NKC])

            emit_qkv_compute(0, xts0)
            xts_n1 = emit_qkv_dma(1)

            # Window w emits its own scores+exp per kc, the ctx groups of
            # window w-1 at kc 0-7 (one per kc, each a closed psum group),
            # phase-A pieces / dense chunks at kc 8-15. Window 0 absorbs
            # phase-A blocks 1-3; n4-n7 ride windows 1-4. dense(b,qb) runs
            # two windows after (b,qb), when its cts is complete.
            order = [(0, 0), (0, 1), (0, 2), (0, 3),
                     (1, 0), (1, 1), (1, 2), (1, 3)]
            # dense spread so neither PE (piece windows, 2 chunks max) nor
            # DVE (staging copies, 7 chunks max) oversubscribes any window
            wdense = {
                2: [(0, 0, 0), (0, 0, 1)],
                3: [(0, 1, 0), (0, 1, 1)],
                4: [(0, 2, 0), (0, 2, 1)],
                5: [(0, 0, 2), (0, 0, 3), (0, 1, 2), (0, 1, 3),
                    (0, 2, 2), (0, 2, 3), (0, 3, 0)],
                6: [(0, 3, 1), (0, 3, 2), (0, 3, 3),
                    (1, 0, 0), (1, 0, 1), (1, 0, 2), (1, 0, 3)],
                7: [(1, 1, 0), (1, 1, 1), (1, 1, 2), (1, 1, 3),
                    (1, 2, 0), (1, 2, 1), (1, 2, 2)],
            }
            prev = None  # (b, qb, ets)
            pending_xts = {1: xts_n1}
            for w, (b, qb) in enumerate(order):
                piece_slots = {}   # kc -> list of phase-A piece indices
                dma_slots = {}
                if w == 0:
                    for i, n in enumerate((1, 2, 3)):
                        for q in range(4):
                            piece_slots[4 * i + q] = (n, q, False)
                    dma_slots = {0: 2, 4: 3, 8: 4}
                elif w <= 4:
                    # fine pieces at kc 8-15: qkv halves adjacent (8,9) and
                    # (11,12); dense pairs at 10 and 13 between closed groups
                    for i, kc_slot in enumerate((8, 9, 11, 12, 14, 15)):
                        piece_slots[kc_slot] = (3 + w, i, True)
                    if w < 4:
                        dma_slots = {6: 4 + w}
                chunks = [(db, dq * 4 + t4) for db, dq, t4 in wdense.get(w, [])]
                dt_slots = {}
                if 1 <= w <= 4:
                    # only the kc10 pair slot is free of open qkv psum groups
                    dt_slots = {10: chunks}
                else:
                    for i, c in enumerate(chunks):
                        dt_slots.setdefault(8 + i % 8, []).append(c)

                ets = {}
                for kc in range(NKC):
                    if prev is not None and kc < 8:
                        pb, pq, pets = prev
                        emit_ctx_group(pb, pq, kc // 4, kc % 4, pets)
                    if kc in piece_slots:
                        # before this kc's scores: window 4's late scores
                        # read kt written by the same-slot n7 piece
                        pn, pq_, fine = piece_slots[kc]
                        pcs = phase_a_pieces(pn, pending_xts[pn], fine=fine)
                        if fine:
                            # 6 slots: m halves (0,1),(2,3) then v pairs
                            fmap = [[0], [1], [2], [3], [4, 5], [6, 7]]
                            for pi in fmap[pq_]:
                                pcs[pi]()
                        else:
                            pcs[pq_]()
                    ets[kc] = emit_scores_exp(b, qb, kc)
                    for ch in dt_slots.get(kc, []):
                        emit_dense_t(*ch)
                    if kc in dma_slots:
                        pending_xts[dma_slots[kc]] = emit_qkv_dma(dma_slots[kc])
                prev = (b, qb, ets)

            pb, pq, pets = prev
            emit_dense_t(1, 2 * 4 + 3, tail=True)  # (1,2) last chunk
            for qc in range(4):
                # both j-groups of a qc, then its dense chunk: the chain
                # pipelines instead of waiting for all 8 groups
                emit_ctx_group(pb, pq, 0, qc, pets)
                emit_ctx_group(pb, pq, 1, qc, pets)
                emit_dense_t(pb, pq * 4 + qc, tail=True)
    nc.compile()
    return nc


def _to_bf16(x):
    import ml_dtypes
    return np.ascontiguousarray(x, dtype=np.float32).astype(ml_dtypes.bfloat16)


def _prepare_inputs(hidden_states, qkv_w, qkv_b, dense_w):
    """Host-side slicing/transposition into per-core input maps."""
    x = np.ascontiguousarray(hidden_states, dtype=np.float32).reshape(BS, H)
    xt = _to_bf16(x.T)
    eye = np.eye(128, dtype=np.float32)
    ones16 = np.ones((128, 16), dtype=np.float32)
    in_maps = []
    for c in range(NCORES):
        h0, h1 = 2 * c, 2 * c + 1
        rows = {}
        for m in range(3):  # 0=q 1=k 2=v; per-head groups of 192 rows
            rows[m] = np.r_[h0 * 192 + m * HD:h0 * 192 + (m + 1) * HD,
                            h1 * 192 + m * HD:h1 * 192 + (m + 1) * HD]
        perm = np.concatenate([rows[0], rows[1], rows[2]])
        w1tc = _to_bf16(qkv_w[perm, :].T)                    # [H, 384]
        b1c = np.ascontiguousarray(
            np.stack([qkv_b[rows[m]] for m in range(2)], axis=1),
            dtype=np.float32)                                # [128, 2]
        w2tc = _to_bf16(dense_w[:, c * 128:(c + 1) * 128].T)  # [128, H]
        in_maps.append({
            "xt": xt, "w1t": w1tc, "b1": b1c, "w2t": w2tc,
            "eye": _to_bf16(eye), "ones16": _to_bf16(ones16),
        })
    return in_maps


def _reference_numpy(hidden_states, attention_mask, qkv_w, qkv_b, dense_w, dense_b):
    """Exact fallback for non-all-ones masks (never hit with spec inputs)."""
    x = np.asarray(hidden_states, dtype=np.float64)
    mask = np.asarray(attention_mask, dtype=np.float64)
    mixed = x @ np.asarray(qkv_w, np.float64).T + np.asarray(qkv_b, np.float64)
    mixed = mixed.reshape(B, S, NH, 3 * HD).transpose(0, 2, 1, 3)
    q, k, v = np.split(mixed, 3, axis=-1)
    scores = np.einsum("bhqd,bhkd->bhqk", q, k) / np.sqrt(HD)
    scores = scores * mask - 10000.0 * (1.0 - mask)
    scores -= scores.max(axis=-1, keepdims=True)
    probs = np.exp(scores)
    probs /= probs.sum(axis=-1, keepdims=True)
    cx = np.einsum("bhqk,bhkd->bhqd", probs, v)
    cx = cx.transpose(0, 2, 1, 3).reshape(B, S, H)
    o = cx @ np.asarray(dense_w, np.float64).T + np.asarray(dense_b, np.float64)
    return o.astype(np.float32)


def _run(inputs, trace=False):
    from concourse.bass_utils import run_bass_kernel_spmd
    if "nc" not in _CACHE:
        _CACHE["nc"] = _build_program()
    nc = _CACHE["nc"]
    in_maps = _prepare_inputs(inputs["hidden_states"], inputs["qkv_w"],
                              inputs["qkv_b"], inputs["dense_w"])
    res = run_bass_kernel_spmd(nc, in_maps, core_ids=list(range(NCORES)),
                               trace=trace)
    partials = np.stack([np.asarray(r["out"], dtype=np.float64)
                         for r in res.results], axis=0)
    full = partials.sum(axis=0)
    # v bias folds into the output bias: ctx = (ctx_raw + bv * rowsum)/rowsum
    qkv_b = np.asarray(inputs["qkv_b"], dtype=np.float64)
    bv = qkv_b.reshape(NH, 3 * HD)[:, 2 * HD:3 * HD].reshape(H)  # ctx col order
    bias_eff = np.asarray(inputs["dense_b"], dtype=np.float64) \
        + np.asarray(inputs["dense_w"], dtype=np.float64) @ bv
    full += bias_eff
    return full.astype(np.float32).reshape(B, S, H), res


def kernel(hidden_states, attention_mask, qkv_w, qkv_b, dense_w, dense_b):
    hidden_states = np.asarray(hidden_states)
    attention_mask = np.asarray(attention_mask)
    qkv_w = np.asarray(qkv_w)
    qkv_b = np.asarray(qkv_b)
    dense_w = np.asarray(dense_w)
    dense_b = np.asarray(dense_b)
    if not np.all(attention_mask == 1.0):
        return _reference_numpy(hidden_states, attention_mask, qkv_w, qkv_b,
                                dense_w, dense_b)
    out, _ = _run({
        "hidden_states": hidden_states, "qkv_w": qkv_w, "qkv_b": qkv_b,
        "dense_w": dense_w, "dense_b": dense_b,
    }, trace=bool(int(os.environ.get("KERNEL_TRACE", "0"))))
    return out
